# Initial kernel scaffold
#
"""Multi-head attention (S=2048, B=2, D=1024, H=16, Hd=64) on 8 trn2 cores.

Sharding: core = (batch b, head-group g of 4 heads)  -> 2*4 = 8 cores.
Each core computes the full attention for its 4 heads / 1 batch and a
partial output projection (row-parallel Wo); the host sums the 4 partials
per batch and adds bo.

Per-core device kernel layout choices:
  - host pre-transposes x -> x^T [D, S] (bf16) so projections contract over
    D on partitions with no on-chip transposes.
  - q^T/k^T are produced pair-packed: [128(e of 2 heads), 2(pair), S] bf16.
  - v is stored per head as v_aug [t, 65] bf16 with column 64 = 1.0; the
    attention matmul (M=65) then yields row 64 = softmax denominator Z.
  - scores are computed transposed ([t, s]) with K=64 row-tiled matmul
    pairs (two heads concurrently in PE row groups 0 and 2).
  - exp on ACT (psum -> sbuf bf16, scale=1/8); softmax normalization via
    reciprocal_approx_fast + DRAM-bounce partition broadcast.
  - output projection in float32r (full-rate fp32) accumulating 4 heads
    per PSUM tile.
"""

import sys

for _p in ("/opt/trn_rl_repo", "/root/.axon_site/_ro/trn_rl_repo"):
    if _p not in sys.path:
        sys.path.insert(0, _p)

import numpy as np
import ml_dtypes

S = 2048
B = 2
D = 1024
H = 16
HD = 64
NH = 4  # heads per core
P = 128
KD = D // P  # 8 contraction tiles for projections

BF16 = ml_dtypes.bfloat16

_BUILD_CACHE = {}


def build_bass(s=S, debug_taps=False):
    """Build the per-core Bass module (same program for all 8 cores)."""
    import concourse.bacc as bacc
    import concourse.bass as bass
    import concourse.mybir as mybir
    import concourse.tile as tile

    f32 = mybir.dt.float32
    f32r = mybir.dt.float32r
    bf16 = mybir.dt.bfloat16
    AF = mybir.ActivationFunctionType
    ALU = mybir.AluOpType

    NT = s // P            # t tiles
    WSC = min(1024, s)     # scores/exp tile width (s columns)
    NSH = s // WSC         # s-half rounds
    CW = min(512, WSC)     # chain width (one psum bank)
    NCH = WSC // CW        # chains per head per round

    nc = bacc.Bacc("TRN2", target_bir_lowering=False, debug=False, num_devices=8)

    xq = nc.dram_tensor("xq_t", [D, s], bf16, kind="ExternalInput").ap()
    xk = nc.dram_tensor("xk_t", [D, s], bf16, kind="ExternalInput").ap()
    xv = nc.dram_tensor("xv_t", [D, s], bf16, kind="ExternalInput").ap()
    wq = nc.dram_tensor("wq_t", [D, 256], bf16, kind="ExternalInput").ap()
    wk = nc.dram_tensor("wk_t", [D, 256], bf16, kind="ExternalInput").ap()
    wv = nc.dram_tensor("wv_t", [D, 256], bf16, kind="ExternalInput").ap()
    wo = nc.dram_tensor("wo_h", [P, 2, D], f32r, kind="ExternalInput").ap()
    bq2 = nc.dram_tensor("bq2", [P, 2], f32, kind="ExternalInput").ap()
    bk2 = nc.dram_tensor("bk2", [P, 2], f32, kind="ExternalInput").ap()
    bv4 = nc.dram_tensor("bv4", [P, 256], f32, kind="ExternalInput").ap()
    out = nc.dram_tensor("out", [s, D], f32, kind="ExternalOutput").ap()

    from contextlib import ExitStack

    with tile.TileContext(nc) as tc, ExitStack() as ctx:
        consts = ctx.enter_context(tc.tile_pool(name="consts", bufs=1))
        persist = ctx.enter_context(tc.tile_pool(name="persist", bufs=1))
        xpool = ctx.enter_context(tc.tile_pool(name="xpool", bufs=2 * KD))
        epool = ctx.enter_context(tc.tile_pool(name="epool", bufs=16))
        rzpool = ctx.enter_context(tc.tile_pool(name="rzpool", bufs=2))
        ospool = ctx.enter_context(tc.tile_pool(name="ospool", bufs=3))
        drampool = ctx.enter_context(tc.tile_pool(name="drampool", bufs=2, space="DRAM"))
        wide = ctx.enter_context(tc.tile_pool(name="wide", bufs=2, space="PSUM"))
        accp = ctx.enter_context(tc.tile_pool(name="accp", bufs=2 * NCH, space="PSUM"))

        # ---- constants -------------------------------------------------
        wq_sb = consts.tile([P, KD, 256], bf16, name="wq_sb")
        nc.sync.dma_start(out=wq_sb, in_=wq.rearrange("(k p) e -> p k e", p=P))
        wk_sb = consts.tile([P, KD, 256], bf16, name="wk_sb")
        nc.sync.dma_start(out=wk_sb, in_=wk.rearrange("(k p) e -> p k e", p=P))
        wv_sb = consts.tile([P, KD, 256], bf16, name="wv_sb")
        nc.sync.dma_start(out=wv_sb, in_=wv.rearrange("(k p) e -> p k e", p=P))
        wo_sb = consts.tile([P, 2, D], f32r, name="wo_sb")
        nc.sync.dma_start(out=wo_sb, in_=wo)
        bq_sb = consts.tile([P, 2], f32, name="bq_sb")
        nc.sync.dma_start(out=bq_sb, in_=bq2)
        bk_sb = consts.tile([P, 2], f32, name="bk_sb")
        nc.sync.dma_start(out=bk_sb, in_=bk2)
        bv_sb = consts.tile([P, 256], f32, name="bv_sb")
        nc.sync.dma_start(out=bv_sb, in_=bv4)

        # ---- persistent activations -----------------------------------
        q2 = persist.tile([P, 2, s], bf16, name="q2")
        k2 = persist.tile([P, 2, s], bf16, name="k2")
        v_aug = persist.tile([P, NH, NT, 65], bf16, name="v_aug")
        nc.vector.memset(v_aug, 1.0)  # col 64 stays 1.0 = Z ones column
        # attn2: pair-packed normalized attention [128(e of 2 heads), 2, s]
        attn2 = persist.tile([P, 2, s], f32r, name="attn2")

        # ---- load x^T and project -------------------------------------
        def load_x(xdram):
            x3 = xdram.rearrange("(k p) s -> k p s", p=P)
            tiles = []
            for k in range(KD):
                xt = xpool.tile([P, s], bf16, tag="x", name=f"xt{k}")
                nc.sync.dma_start(out=xt, in_=x3[k])
                tiles.append(xt)
            return tiles

        def proj_round(xtiles, w_sb, b_sb, dst, p, sh):
            # dst[:, p, sh-slice] = ((x @ W_pair.T)^T + bias) for one s-half
            ps = wide.tile([P, WSC], f32, tag="wide", name="qkps")
            for c in range(NCH):
                for k in range(KD):
                    nc.tensor.matmul(
                        ps[:, c * CW:(c + 1) * CW],
                        lhsT=w_sb[:, k, p * P:(p + 1) * P],
                        rhs=xtiles[k][:, sh * WSC + c * CW: sh * WSC + (c + 1) * CW],
                        start=(k == 0),
                        stop=(k == KD - 1),
                    )
            nc.vector.tensor_scalar(
                dst[:, p, sh * WSC:(sh + 1) * WSC], ps, b_sb[:, p:p + 1],
                None, ALU.add,
            )

        def v_round(xtiles, t):
            ps = wide.tile([P, 256], f32, tag="wide", name="vps")
            for k in range(KD):
                nc.tensor.matmul(
                    ps,
                    lhsT=xtiles[k][:, t * P:(t + 1) * P],
                    rhs=wv_sb[:, k, :],
                    start=(k == 0),
                    stop=(k == KD - 1),
                )
            for h in range(NH):
                nc.vector.tensor_tensor(
                    v_aug[:, h, t, 0:64],
                    ps[:, h * 64:(h + 1) * 64],
                    bv_sb[:, h * 64:(h + 1) * 64],
                    ALU.add,
                )

        xq_tiles = load_x(xq)
        for p in range(2):
            for sh in range(NSH):
                proj_round(xq_tiles, wq_sb, bq_sb, q2, p, sh)
        xk_tiles = load_x(xk)
        for p in range(2):
            for sh in range(NSH):
                proj_round(xk_tiles, wk_sb, bk_sb, k2, p, sh)
        xv_tiles = load_x(xv)
        for t in range(NT):
            v_round(xv_tiles, t)

        def out_proj(sc_i):
            op = wide.tile([P, D], f32, tag="wide", name="op")
            for nh_i in range(2):
                for p in range(2):
                    nc.tensor.matmul(
                        op[:, nh_i * 512:(nh_i + 1) * 512],
                        lhsT=attn2[:, p, sc_i * P:(sc_i + 1) * P],
                        rhs=wo_sb[:, p, nh_i * 512:(nh_i + 1) * 512],
                        start=(p == 0),
                        stop=(p == 1),
                    )
            ob = ospool.tile([P, D], f32, tag="ob", name="ob")
            nc.vector.tensor_copy(ob, op)
            nc.sync.dma_start(out=out[sc_i * P:(sc_i + 1) * P, :], in_=ob)

        def normalize(p, hi, soff, chains):
            # attn = attn~ / Z ; Z sits in row 64 of each chain
            rz = rzpool.tile([P, WSC], f32, tag="rz", name="rz")
            for c in range(NCH):
                nc.vector.tensor_copy(
                    rz[64:65, c * CW:(c + 1) * CW],
                    chains[c][64:65, :],
                )
            zd = drampool.tile([1, WSC], f32, tag="zd", name="zd")
            nc.sync.dma_start(out=zd, in_=rz[64:65, :])
            zbc = bass.AP(
                tensor=zd.tensor,
                offset=zd.offset,
                ap=[[0, 64]] + list(zd.ap[-1:]),
            )
            nc.sync.dma_start(out=rz[0:64, :], in_=zbc)
            # reciprocal at base partition 0 (base 64 miscomputes on HW)
            nc.vector.reciprocal_approx_fast(rz[0:64, :], rz[0:64, :])
            if hi == 0:
                # even head of pair -> attn2 rows 0:64 directly
                for c in range(NCH):
                    nc.vector.tensor_tensor(
                        attn2[0:64, p, soff + c * CW: soff + (c + 1) * CW],
                        chains[c][0:64, :],
                        rz[0:64, c * CW:(c + 1) * CW],
                        ALU.mult,
                    )
            else:
                # odd head: drain to tmp then DMA-shift to rows 64:128
                atmp = rzpool.tile([HD, WSC], f32r, tag="atmp", name="atmp")
                for c in range(NCH):
                    nc.vector.tensor_tensor(
                        atmp[:, c * CW:(c + 1) * CW],
                        chains[c][0:64, :],
                        rz[0:64, c * CW:(c + 1) * CW],
                        ALU.mult,
                    )
                nc.sync.dma_start(
                    out=attn2[64:128, p, soff:soff + WSC], in_=atmp
                )

        for sh in range(NSH):
            soff = sh * WSC
            for p in range(2):
                heads = (2 * p, 2 * p + 1)
                chains = [
                    [accp.tile([P, CW], f32, tag="chain", name=f"ch{hi}_{c}")
                     for c in range(NCH)]
                    for hi in range(2)
                ]
                for t in range(NT):
                    etiles = []
                    for hi in range(2):
                        rlo, rhi = (0, 64) if hi == 0 else (64, 128)
                        sc = wide.tile([P, WSC], f32, tag="wide", name=f"sc{hi}")
                        for c in range(NCH):
                            nc.tensor.matmul(
                                sc[:, c * CW:(c + 1) * CW],
                                lhsT=k2[rlo:rhi, p, t * P:(t + 1) * P],
                                rhs=q2[rlo:rhi, p, soff + c * CW: soff + (c + 1) * CW],
                                start=True,
                                stop=True,
                                tile_position=(rlo, 0),
                            )
                        et = epool.tile([P, WSC], bf16, tag="exp", name=f"exp{hi}")
                        nc.scalar.activation(et, sc, AF.Exp, bias=0.0, scale=0.125)
                        etiles.append(et)
                    for hi in range(2):
                        for c in range(NCH):
                            nc.tensor.matmul(
                                chains[hi][c][0:65, :],
                                lhsT=v_aug[:, heads[hi], t, :],
                                rhs=etiles[hi][:, c * CW:(c + 1) * CW],
                                start=(t == 0),
                                stop=(t == NT - 1),
                            )
                normalize(p, 0, soff, chains[0])
                normalize(p, 1, soff, chains[1])

        for sc_i in range(s // P):
            out_proj(sc_i)

        if debug_taps:
            dq2 = nc.dram_tensor("dbg_q2", [P, 2, s], bf16, kind="ExternalOutput").ap()
            nc.sync.dma_start(out=dq2, in_=q2)
            dk2 = nc.dram_tensor("dbg_k2", [P, 2, s], bf16, kind="ExternalOutput").ap()
            nc.sync.dma_start(out=dk2, in_=k2)
            dva = nc.dram_tensor("dbg_vaug", [P, NH, NT, 65], bf16, kind="ExternalOutput").ap()
            nc.sync.dma_start(out=dva, in_=v_aug)
            dat = nc.dram_tensor("dbg_attn", [P, 2, s], f32, kind="ExternalOutput").ap()
            nc.sync.dma_start(out=dat, in_=attn2.bitcast(f32))

    nc.compile()
    return nc


def get_bass(s=S):
    if s not in _BUILD_CACHE:
        _BUILD_CACHE[s] = build_bass(s)
    return _BUILD_CACHE[s]


def make_in_maps(query, key, value, Wq, bq, Wk, bk, Wv, bv, Wo):
    """Host-side sharding: per-core input dict for core = b*4 + g."""
    in_maps = []
    for core in range(8):
        b, g = core // 4, core % 4
        cs = slice(g * 256, (g + 1) * 256)
        # pair-packed: wo_h[hd + 64*(h%2), h//2, :] = Wo[:, g*256 + h*64 + hd]
        wo_h = (
            np.ascontiguousarray(Wo[:, cs].T)  # [256(h*64+hd), 1024]
            .reshape(2, P, D)
            .transpose(1, 0, 2)
        )
        m = {
            "xq_t": np.ascontiguousarray(query[:, b, :].T).astype(BF16),
            "xk_t": np.ascontiguousarray(key[:, b, :].T).astype(BF16),
            "xv_t": np.ascontiguousarray(value[:, b, :].T).astype(BF16),
            "wq_t": np.ascontiguousarray(Wq[cs, :].T).astype(BF16),
            "wk_t": np.ascontiguousarray(Wk[cs, :].T).astype(BF16),
            "wv_t": np.ascontiguousarray(Wv[cs, :].T).astype(BF16),
            "wo_h": np.ascontiguousarray(wo_h).astype(np.float32),
            "bq2": np.ascontiguousarray(bq[cs].reshape(2, P).T).astype(np.float32),
            "bk2": np.ascontiguousarray(bk[cs].reshape(2, P).T).astype(np.float32),
            "bv4": np.ascontiguousarray(
                np.broadcast_to(bv[cs], (P, 256))
            ).astype(np.float32),
        }
        in_maps.append(m)
    return in_maps


def kernel(query, key, value, Wq, bq, Wk, bk, Wv, bv, Wo, bo):
    from concourse.bass_utils import run_bass_kernel_spmd

    query = np.asarray(query, dtype=np.float32)
    key = np.asarray(key, dtype=np.float32)
    value = np.asarray(value, dtype=np.float32)
    Wq = np.asarray(Wq, dtype=np.float32)
    Wk = np.asarray(Wk, dtype=np.float32)
    Wv = np.asarray(Wv, dtype=np.float32)
    Wo = np.asarray(Wo, dtype=np.float32)

    nc = get_bass(S)
    in_maps = make_in_maps(query, key, value, Wq, bq, Wk, bk, Wv, bv, Wo)
    res = run_bass_kernel_spmd(nc, in_maps, core_ids=list(range(8)))
    outs = [res.results[c]["out"] for c in range(8)]

    full = np.empty((S, B, D), dtype=np.float32)
    bo32 = np.asarray(bo, dtype=np.float32)
    for b in range(B):
        acc = outs[b * 4].astype(np.float32).copy()
        for g in range(1, 4):
            acc += outs[b * 4 + g]
        full[:, b, :] = acc + bo32[None, :]
    return full



# revision 1
# speedup vs baseline: 1.3592x; 1.3592x over previous
"""Multi-head attention (S=2048, B=2, D=1024, H=16, Hd=64) on 8 trn2 cores.

Sharding: core = (batch b, head-group g of 4 heads)  -> 2*4 = 8 cores.
Each core computes the full attention for its 4 heads / 1 batch and a
partial output projection (row-parallel Wo); the host sums the 4 partials
per batch and adds bo.

Per-core device kernel layout choices:
  - host pre-transposes x -> x^T [D, S] (bf16) so projections contract over
    D on partitions with no on-chip transposes.
  - q^T/k^T are produced pair-packed: [128(e of 2 heads), 2(pair), S] bf16.
  - v is stored per head as v_aug [t, 65] bf16 with column 64 = 1.0; the
    attention matmul (M=65) then yields row 64 = softmax denominator Z.
  - scores are computed transposed ([t, s]) with K=64 row-tiled matmul
    pairs (two heads concurrently in PE row groups 0 and 2).
  - exp on ACT (psum -> sbuf bf16, scale=1/8); softmax normalization via
    reciprocal_approx_fast + DRAM-bounce partition broadcast.
  - output projection in float32r (full-rate fp32) accumulating 4 heads
    per PSUM tile.
"""

import sys

for _p in ("/opt/trn_rl_repo", "/root/.axon_site/_ro/trn_rl_repo"):
    if _p not in sys.path:
        sys.path.insert(0, _p)

import numpy as np
import ml_dtypes

S = 2048
B = 2
D = 1024
H = 16
HD = 64
NH = 4  # heads per core
P = 128
KD = D // P  # 8 contraction tiles for projections

BF16 = ml_dtypes.bfloat16

_BUILD_CACHE = {}


def build_bass(s=S, debug_taps=False):
    """Build the per-core Bass module (same program for all 8 cores)."""
    import concourse.bacc as bacc
    import concourse.bass as bass
    import concourse.mybir as mybir
    import concourse.tile as tile

    f32 = mybir.dt.float32
    f32r = mybir.dt.float32r
    bf16 = mybir.dt.bfloat16
    AF = mybir.ActivationFunctionType
    ALU = mybir.AluOpType

    NT = s // P            # t tiles
    WSC = min(1024, s)     # scores/exp tile width (s columns)
    NSH = s // WSC         # s-half rounds
    CW = min(512, WSC)     # chain width (one psum bank)
    NCH = WSC // CW        # chains per head per round

    nc = bacc.Bacc("TRN2", target_bir_lowering=False, debug=False, num_devices=8)

    xq = nc.dram_tensor("xq_t", [D, s], bf16, kind="ExternalInput").ap()
    xk = nc.dram_tensor("xk_t", [D, s], bf16, kind="ExternalInput").ap()
    xv = nc.dram_tensor("xv_t", [D, s], bf16, kind="ExternalInput").ap()
    wq = nc.dram_tensor("wq_t", [D, 256], bf16, kind="ExternalInput").ap()
    wk = nc.dram_tensor("wk_t", [D, 256], bf16, kind="ExternalInput").ap()
    wv = nc.dram_tensor("wv_t", [D, 256], bf16, kind="ExternalInput").ap()
    wo = nc.dram_tensor("wo_h", [P, 2, D], f32r, kind="ExternalInput").ap()
    bq2 = nc.dram_tensor("bq2", [P, 2], f32, kind="ExternalInput").ap()
    bk2 = nc.dram_tensor("bk2", [P, 2], f32, kind="ExternalInput").ap()
    bv4 = nc.dram_tensor("bv4", [P, 256], f32, kind="ExternalInput").ap()
    out = nc.dram_tensor("out", [s, D], f32, kind="ExternalOutput").ap()

    from contextlib import ExitStack

    with tile.TileContext(nc) as tc, ExitStack() as ctx:
        consts = ctx.enter_context(tc.tile_pool(name="consts", bufs=1))
        persist = ctx.enter_context(tc.tile_pool(name="persist", bufs=1))
        xpool = ctx.enter_context(tc.tile_pool(name="xpool", bufs=2 * KD))
        epool = ctx.enter_context(tc.tile_pool(name="epool", bufs=16))
        rzpool = ctx.enter_context(tc.tile_pool(name="rzpool", bufs=2))
        ospool = ctx.enter_context(tc.tile_pool(name="ospool", bufs=3))
        drampool = ctx.enter_context(tc.tile_pool(name="drampool", bufs=2, space="DRAM"))
        wide = ctx.enter_context(tc.tile_pool(name="wide", bufs=2, space="PSUM"))
        accp = ctx.enter_context(tc.tile_pool(name="accp", bufs=2 * NCH, space="PSUM"))

        # ---- constants -------------------------------------------------
        wq_sb = consts.tile([P, KD, 256], bf16, name="wq_sb")
        nc.sync.dma_start(out=wq_sb, in_=wq.rearrange("(k p) e -> p k e", p=P))
        wk_sb = consts.tile([P, KD, 256], bf16, name="wk_sb")
        nc.sync.dma_start(out=wk_sb, in_=wk.rearrange("(k p) e -> p k e", p=P))
        wv_sb = consts.tile([P, KD, 256], bf16, name="wv_sb")
        nc.sync.dma_start(out=wv_sb, in_=wv.rearrange("(k p) e -> p k e", p=P))
        wo_sb = consts.tile([P, 2, D], f32r, name="wo_sb")
        nc.sync.dma_start(out=wo_sb, in_=wo)
        bq_sb = consts.tile([P, 2], f32, name="bq_sb")
        nc.sync.dma_start(out=bq_sb, in_=bq2)
        bk_sb = consts.tile([P, 2], f32, name="bk_sb")
        nc.sync.dma_start(out=bk_sb, in_=bk2)
        bv_sb = consts.tile([P, 256], f32, name="bv_sb")
        nc.sync.dma_start(out=bv_sb, in_=bv4)

        # ---- persistent activations -----------------------------------
        q2 = persist.tile([P, 2, s], bf16, name="q2")
        k2 = persist.tile([P, 2, s], bf16, name="k2")
        v_aug = persist.tile([P, NH, NT, 65], bf16, name="v_aug")
        nc.vector.memset(v_aug, 1.0)  # col 64 stays 1.0 = Z ones column
        # attn2: pair-packed normalized attention [128(e of 2 heads), 2, s]
        attn2 = persist.tile([P, 2, s], f32r, name="attn2")

        # ---- load x^T and project -------------------------------------
        def load_x(xdram):
            x3 = xdram.rearrange("(k p) s -> k p s", p=P)
            tiles = []
            for k in range(KD):
                xt = xpool.tile([P, s], bf16, tag="x", name=f"xt{k}")
                nc.sync.dma_start(out=xt, in_=x3[k])
                tiles.append(xt)
            return tiles

        def proj_round(xtiles, w_sb, b_sb, dst, p, sh):
            # dst[:, p, sh-slice] = ((x @ W_pair.T)^T + bias) for one s-half
            ps = wide.tile([P, WSC], f32, tag="wide", name="qkps")
            for c in range(NCH):
                for k in range(KD):
                    nc.tensor.matmul(
                        ps[:, c * CW:(c + 1) * CW],
                        lhsT=w_sb[:, k, p * P:(p + 1) * P],
                        rhs=xtiles[k][:, sh * WSC + c * CW: sh * WSC + (c + 1) * CW],
                        start=(k == 0),
                        stop=(k == KD - 1),
                    )
            nc.vector.tensor_scalar(
                dst[:, p, sh * WSC:(sh + 1) * WSC], ps, b_sb[:, p:p + 1],
                None, ALU.add,
            )

        def v_round(xtiles, t):
            ps = wide.tile([P, 256], f32, tag="wide", name="vps")
            for k in range(KD):
                nc.tensor.matmul(
                    ps,
                    lhsT=xtiles[k][:, t * P:(t + 1) * P],
                    rhs=wv_sb[:, k, :],
                    start=(k == 0),
                    stop=(k == KD - 1),
                )
            for h in range(NH):
                nc.vector.tensor_tensor(
                    v_aug[:, h, t, 0:64],
                    ps[:, h * 64:(h + 1) * 64],
                    bv_sb[:, h * 64:(h + 1) * 64],
                    ALU.add,
                )

        xq_tiles = load_x(xq)
        for p in range(2):
            for sh in range(NSH):
                proj_round(xq_tiles, wq_sb, bq_sb, q2, p, sh)
        xk_tiles = load_x(xk)
        for p in range(2):
            for sh in range(NSH):
                proj_round(xk_tiles, wk_sb, bk_sb, k2, p, sh)
        xv_tiles = load_x(xv)
        for t in range(NT):
            v_round(xv_tiles, t)

        def out_proj(sc_i):
            op = wide.tile([P, D], f32, tag="wide", name="op")
            for nh_i in range(2):
                for p in range(2):
                    nc.tensor.matmul(
                        op[:, nh_i * 512:(nh_i + 1) * 512],
                        lhsT=attn2[:, p, sc_i * P:(sc_i + 1) * P],
                        rhs=wo_sb[:, p, nh_i * 512:(nh_i + 1) * 512],
                        start=(p == 0),
                        stop=(p == 1),
                    )
            ob = ospool.tile([P, D], f32, tag="ob", name="ob")
            nc.vector.tensor_copy(ob, op)
            nc.sync.dma_start(out=out[sc_i * P:(sc_i + 1) * P, :], in_=ob)

        def normalize(p, hi, soff, chains):
            # attn = attn~ / Z ; Z sits in row 64 of each chain
            rz = rzpool.tile([P, WSC], f32, tag="rz", name="rz")
            for c in range(NCH):
                nc.vector.tensor_copy(
                    rz[64:65, c * CW:(c + 1) * CW],
                    chains[c][64:65, :],
                )
            zd = drampool.tile([1, WSC], f32, tag="zd", name="zd")
            nc.sync.dma_start(out=zd, in_=rz[64:65, :])
            zbc = bass.AP(
                tensor=zd.tensor,
                offset=zd.offset,
                ap=[[0, 64]] + list(zd.ap[-1:]),
            )
            nc.sync.dma_start(out=rz[0:64, :], in_=zbc)
            # reciprocal at base partition 0 (base 64 miscomputes on HW)
            nc.vector.reciprocal_approx_fast(rz[0:64, :], rz[0:64, :])
            if hi == 0:
                # even head of pair -> attn2 rows 0:64 directly
                for c in range(NCH):
                    nc.vector.tensor_tensor(
                        attn2[0:64, p, soff + c * CW: soff + (c + 1) * CW],
                        chains[c][0:64, :],
                        rz[0:64, c * CW:(c + 1) * CW],
                        ALU.mult,
                    )
            else:
                # odd head: drain to tmp then DMA-shift to rows 64:128
                atmp = rzpool.tile([HD, WSC], f32r, tag="atmp", name="atmp")
                for c in range(NCH):
                    nc.vector.tensor_tensor(
                        atmp[:, c * CW:(c + 1) * CW],
                        chains[c][0:64, :],
                        rz[0:64, c * CW:(c + 1) * CW],
                        ALU.mult,
                    )
                nc.sync.dma_start(
                    out=attn2[64:128, p, soff:soff + WSC], in_=atmp
                )

        for sh in range(NSH):
            soff = sh * WSC
            for p in range(2):
                heads = (2 * p, 2 * p + 1)
                chains = [
                    [accp.tile([P, CW], f32, tag="chain", name=f"ch{hi}_{c}")
                     for c in range(NCH)]
                    for hi in range(2)
                ]
                for t in range(NT):
                    etiles = []
                    for hi in range(2):
                        rlo, rhi = (0, 64) if hi == 0 else (64, 128)
                        sc = wide.tile([P, WSC], f32, tag="wide", name=f"sc{hi}")
                        for c in range(NCH):
                            nc.tensor.matmul(
                                sc[:, c * CW:(c + 1) * CW],
                                lhsT=k2[rlo:rhi, p, t * P:(t + 1) * P],
                                rhs=q2[rlo:rhi, p, soff + c * CW: soff + (c + 1) * CW],
                                start=True,
                                stop=True,
                                tile_position=(rlo, 0),
                            )
                        et = epool.tile([P, WSC], bf16, tag="exp", name=f"exp{hi}")
                        nc.scalar.activation(et, sc, AF.Exp, bias=0.0, scale=0.125)
                        etiles.append(et)
                    for hi in range(2):
                        for c in range(NCH):
                            nc.tensor.matmul(
                                chains[hi][c][0:65, :],
                                lhsT=v_aug[:, heads[hi], t, :],
                                rhs=etiles[hi][:, c * CW:(c + 1) * CW],
                                start=(t == 0),
                                stop=(t == NT - 1),
                            )
                normalize(p, 0, soff, chains[0])
                normalize(p, 1, soff, chains[1])

        for sc_i in range(s // P):
            out_proj(sc_i)

        if debug_taps:
            dq2 = nc.dram_tensor("dbg_q2", [P, 2, s], bf16, kind="ExternalOutput").ap()
            nc.sync.dma_start(out=dq2, in_=q2)
            dk2 = nc.dram_tensor("dbg_k2", [P, 2, s], bf16, kind="ExternalOutput").ap()
            nc.sync.dma_start(out=dk2, in_=k2)
            dva = nc.dram_tensor("dbg_vaug", [P, NH, NT, 65], bf16, kind="ExternalOutput").ap()
            nc.sync.dma_start(out=dva, in_=v_aug)
            dat = nc.dram_tensor("dbg_attn", [P, 2, s], f32, kind="ExternalOutput").ap()
            nc.sync.dma_start(out=dat, in_=attn2.bitcast(f32))

    nc.compile()
    return nc


def get_bass(s=S):
    if s not in _BUILD_CACHE:
        _BUILD_CACHE[s] = build_bass(s)
    return _BUILD_CACHE[s]


def make_in_maps(query, key, value, Wq, bq, Wk, bk, Wv, bv, Wo):
    """Host-side sharding: per-core input dict for core = b*4 + g."""
    in_maps = []
    for core in range(8):
        b, g = core // 4, core % 4
        cs = slice(g * 256, (g + 1) * 256)
        # pair-packed: wo_h[hd + 64*(h%2), h//2, :] = Wo[:, g*256 + h*64 + hd]
        wo_h = (
            np.ascontiguousarray(Wo[:, cs].T)  # [256(h*64+hd), 1024]
            .reshape(2, P, D)
            .transpose(1, 0, 2)
        )
        m = {
            "xq_t": np.ascontiguousarray(query[:, b, :].T).astype(BF16),
            "xk_t": np.ascontiguousarray(key[:, b, :].T).astype(BF16),
            "xv_t": np.ascontiguousarray(value[:, b, :].T).astype(BF16),
            "wq_t": np.ascontiguousarray(Wq[cs, :].T).astype(BF16),
            "wk_t": np.ascontiguousarray(Wk[cs, :].T).astype(BF16),
            "wv_t": np.ascontiguousarray(Wv[cs, :].T).astype(BF16),
            "wo_h": np.ascontiguousarray(wo_h).astype(np.float32),
            "bq2": np.ascontiguousarray(bq[cs].reshape(2, P).T).astype(np.float32),
            "bk2": np.ascontiguousarray(bk[cs].reshape(2, P).T).astype(np.float32),
            "bv4": np.ascontiguousarray(
                np.broadcast_to(bv[cs], (P, 256))
            ).astype(np.float32),
        }
        in_maps.append(m)
    return in_maps


def kernel(query, key, value, Wq, bq, Wk, bk, Wv, bv, Wo, bo):
    from concourse.bass_utils import run_bass_kernel_spmd

    query = np.asarray(query, dtype=np.float32)
    key = np.asarray(key, dtype=np.float32)
    value = np.asarray(value, dtype=np.float32)
    Wq = np.asarray(Wq, dtype=np.float32)
    Wk = np.asarray(Wk, dtype=np.float32)
    Wv = np.asarray(Wv, dtype=np.float32)
    Wo = np.asarray(Wo, dtype=np.float32)

    nc = get_bass(S)
    in_maps = make_in_maps(query, key, value, Wq, bq, Wk, bk, Wv, bv, Wo)
    res = run_bass_kernel_spmd(nc, in_maps, core_ids=list(range(8)))
    outs = [res.results[c]["out"] for c in range(8)]

    full = np.empty((S, B, D), dtype=np.float32)
    bo32 = np.asarray(bo, dtype=np.float32)
    for b in range(B):
        acc = outs[b * 4].astype(np.float32).copy()
        for g in range(1, 4):
            acc += outs[b * 4 + g]
        full[:, b, :] = acc + bo32[None, :]
    return full

